# revision 2
# baseline (speedup 1.0000x reference)
"""AvgDistanceConv (GNN message passing) on 8 Trainium2 NeuronCores — v2.

out[:, 0] = pos = h[:, 0]
out[:, 1] = segment_mean over incoming edges of |pos[src] - pos[dst]|

v2 strategy (vs v1's per-column indirect DMA, 1.1us per 128 gathered values):
dst-range sharding as before, but the gather runs through the SWDGE
dma_gather ucode (InstDMAGatherAnt), which generates one 8-byte descriptor
per edge slot at ~0.34ns/descriptor after a ~1us fixed cost per call.
Empirically a call is limited to 1024 indices (65 descriptors per DMA ring),
still amortizing the fixed cost 8x better than indirect DMA.

dma_gather indices are int16 (<32768), so pos is staged 4-packed:
row j of a [25000, 128]-bf16 table (256B stride) holds
(pos[j], pos[j+25000], pos[j+50000], pos[j+75000]) in cols 0..3; each
descriptor fetches all 4 lanes (8B) and a host-staged bf16 one-hot picks the
right lane on DVE (also zeroing pad slots):
  s_row = reduce_abs_add( (g4 - pos_dst) * onehot )
since onehot ∈ {0,1} ⇒ |d|*oh = |d*oh|.

Slot mapping (from the ucode): out[p, c, :] = table[idx_stream[c*128+p], 0:4]
with idx_stream[m] = idxs_sbuf[m%16, m//16] (wrapped-16, replicated to 128
partitions).
"""
import sys
sys.path.insert(0, '/opt/trn_rl_repo')
import numpy as np
import ml_dtypes
import concourse.bass as bass
import concourse.mybir as mybir
from concourse.bass_utils import run_bass_kernel_spmd
from concourse.tile import TileContext
from concourse import library_config

P = 128
NC = 8
N_NODES = 100000
RLANES = 4
RROWS = N_NODES // RLANES      # 25000 rows in the packed pos table
ROWW = 128                     # bf16 per row -> 256B stride
CALL_IDXS = 1024               # 65 descs/ring (proven safe w/ 4 queues)
CALL_COLS = CALL_IDXS // P     # 12 ELL columns per gather call
IDX_GROUP_CALLS = 128          # idx staging granularity (128 calls / group)


def _split_sync_waits(nc, max_waits=1):
    """This walrus build rejects more than one sync wait per instruction.
    Hoist extras into standalone same-engine EventSemaphore waits placed
    immediately before the owning instruction (same-engine program order
    preserves the synchronization semantics)."""
    for f in nc.m.functions:
        for blk in f.blocks:
            insts = list(blk.instructions)
            new = []
            dirty = False
            for inst in insts:
                si = inst.sync_info
                if si is not None and len(si.on_wait) > max_waits:
                    waits = list(si.on_wait)
                    for j, w in enumerate(waits[:-max_waits]):
                        wi = mybir.InstEventSemaphore(
                            name=f"{inst.name}_hw{j}", ins=[], outs=[])
                        wi.engine = inst.engine
                        wi.sync_info = mybir.SyncInfo(on_wait=[w], on_update=[])
                        new.append(wi)
                    inst.sync_info = mybir.SyncInfo(
                        on_wait=waits[-max_waits:], on_update=list(si.on_update))
                    dirty = True
                new.append(inst)
            if dirty:
                blk.instructions = new


def _dma_gather_raw(nc, out_ap, in_ap, idxs_ap, num_idxs, elem_size, elem_step,
                    nreg, queue_num=0):
    """InstDMAGatherAnt emitter: bass.dma_gather minus the 256B elem_size
    assert (the ucode only requires the row STRIDE to be a 256B multiple in
    non-transpose HBM-source mode)."""
    eng = nc.gpsimd
    stride_bytes = elem_step * mybir.dt.size(in_ap.dtype)
    assert stride_bytes % 256 == 0 and stride_bytes // 256 < 256
    assert in_ap.ap[0][0] == elem_step
    _in_ap = eng.lower_ap_dma(in_ap, for_custom_bir_dma=True)
    _idxs_ap = eng.lower_ap(idxs_ap)
    _out_ap = eng.lower_ap(out_ap)
    return eng.add_instruction(
        mybir.InstDMAGatherAnt(
            name=nc.get_next_instruction_name(),
            ins=[*_in_ap, _idxs_ap, eng.lower_val_access(nreg)],
            outs=[_out_ap],
            transpose=False,
            num_idxs=num_idxs,
            elem_size=elem_size,
            stride_bytes_256=stride_bytes // 256,
            gen_mode=0,
            single_packet=True,
            queue_num=queue_num,
            sbuf_tokens_per_rank=0,
            sbuf_free_dim_per_rank=0,
            sbuf_free_dim_pad_per_rank=0,
            sbuf_byte_offset=0,
        ))


def _host_prep(h, src, dst):
    N = N_NODES
    NPC = N // NC                      # 12500 dst nodes per core
    TILES = (NPC + P - 1) // P         # 98
    ROWS = TILES * P                   # 12544
    E = src.shape[0]

    pos = np.ascontiguousarray(h[:, 0], dtype=np.float32)
    src32 = src.astype(np.int32)
    dst32 = dst.astype(np.int32)

    # 4-packed bf16 pos table, 256B row stride (shared by all cores)
    pos_rows = np.zeros((RROWS, ROWW), dtype=ml_dtypes.bfloat16)
    for q in range(RLANES):
        pos_rows[:, q] = pos[q * RROWS:(q + 1) * RROWS].astype(ml_dtypes.bfloat16)

    cnt = np.bincount(dst32, minlength=N)
    order = np.argsort(dst32, kind='stable')
    ssrc = src32[order]
    starts = np.zeros(N + 1, np.int64)
    starts[1:] = np.cumsum(cnt)

    deg_c = cnt.reshape(NC, NPC)
    rank = np.argsort(-deg_c, axis=1, kind='stable')
    node_ids = rank + (np.arange(NC)[:, None] * NPC)
    deg_sorted = np.take_along_axis(deg_c, rank, axis=1)

    pad = ROWS - NPC
    node_ids_p = np.concatenate(
        [node_ids, np.repeat(np.arange(NC)[:, None] * NPC, pad, axis=1)], axis=1)
    deg_p = np.concatenate([deg_sorted, np.zeros((NC, pad), np.int64)], axis=1)

    # per-tile slot width, shared across cores (SPMD: one program for all)
    K_t = np.maximum(deg_p.reshape(NC, TILES, P).max(axis=(0, 2)), 1).astype(int)
    COLS = int(K_t.sum())
    # pad COLS so the call count divides evenly into whole calls
    COLS_PAD = ((COLS + CALL_COLS - 1) // CALL_COLS) * CALL_COLS
    NCALLS = COLS_PAD // CALL_COLS

    Kmax = int(K_t.max())
    ar = np.arange(Kmax)
    slot_idx = starts[node_ids_p][:, :, None] + ar[None, None, :]
    valid = ar[None, None, :] < deg_p[:, :, None]
    ell = np.where(valid, ssrc[np.minimum(slot_idx, E - 1)], 0).astype(np.int32)

    # ELL as [NC, P, COLS_PAD] column-major over tiles + validity mask
    ell_w = np.zeros((NC, P, COLS_PAD), np.int32)
    msk_w = np.zeros((NC, P, COLS_PAD), bool)
    off = 0
    col_tile = np.zeros(COLS_PAD, np.int32)       # tile id per column
    for t in range(TILES):
        K = int(K_t[t])
        r0 = t * P
        ell_w[:, :, off:off + K] = ell[:, r0:r0 + P, :K].transpose(0, 2, 1) \
            .transpose(0, 2, 1)
        ell_w[:, :, off:off + K] = ell[:, r0:r0 + P, :K]
        msk_w[:, :, off:off + K] = valid[:, r0:r0 + P, :K]
        col_tile[off:off + K] = t
        off += K
    col_tile[off:] = TILES  # trailing pad columns belong to no tile

    lid = (ell_w % RROWS).astype(np.int16)        # row in packed table
    lane = (ell_w // RROWS).astype(np.int8)       # lane 0..3

    # idx stream per core: stream[c*128 + p] = lid[core, p, c]
    # wrapped-16: idxs_arr[pp, f] = stream[f*16 + pp], replicated to 128
    stream = lid.transpose(0, 2, 1).reshape(NC, COLS_PAD * P)   # [NC, S]
    wrapped = stream.reshape(NC, -1, 16).transpose(0, 2, 1)     # [NC, 16, S/16]
    idx_rep = np.tile(wrapped, (1, 8, 1)).astype(np.int16)      # [NC, 128, S/16]

    # bf16 one-hot [P, 4*COLS_PAD]: oh[p, 4c+q] = (lane==q and valid)
    oh = np.zeros((NC, P, COLS_PAD, RLANES), dtype=ml_dtypes.bfloat16)
    np.put_along_axis(
        oh, lane[..., None].astype(np.int64),
        msk_w[..., None].astype(ml_dtypes.bfloat16), axis=3)
    oh = oh.reshape(NC, P, COLS_PAD * RLANES)

    cntf = deg_p.astype(np.float32)
    posr = pos[node_ids_p].astype(np.float32)     # [NC, ROWS] dst pos (f32)
    posrb = posr.astype(ml_dtypes.bfloat16)       # bf16 for the subtract

    # [P, TILES] layouts for the reduce/divide stage
    posr_pt = posr.reshape(NC, TILES, P).transpose(0, 2, 1)
    posrb_pt = posrb.reshape(NC, TILES, P).transpose(0, 2, 1)
    cnt_pt = cntf.reshape(NC, TILES, P).transpose(0, 2, 1)

    in_maps = []
    for c in range(NC):
        in_maps.append({
            "posrows": pos_rows,
            "idx": idx_rep[c],
            "oh": np.ascontiguousarray(oh[c]),
            "posr": np.ascontiguousarray(posr_pt[c]),
            "negposr": np.ascontiguousarray(-posr_pt[c]),
            "cntf": np.ascontiguousarray(cnt_pt[c]),
        })
    meta = dict(N=N, NPC=NPC, TILES=TILES, ROWS=ROWS, K_t=K_t,
                COLS_PAD=COLS_PAD, NCALLS=NCALLS, node_ids=node_ids)
    return in_maps, meta


def _build_program(meta):
    TILES, ROWS, K_t, COLS_PAD, NCALLS = (meta["TILES"], meta["ROWS"],
                                          meta["K_t"], meta["COLS_PAD"],
                                          meta["NCALLS"])
    S = COLS_PAD * P
    nc = bass.Bass(num_swdge_queues=4)
    posrows = nc.declare_dram_parameter("posrows", [RROWS, ROWW],
                                        mybir.dt.bfloat16, isOutput=False)
    idx_d = nc.declare_dram_parameter("idx", [P, S // 16], mybir.dt.int16,
                                      isOutput=False)
    oh_d = nc.declare_dram_parameter("oh", [P, COLS_PAD * RLANES],
                                     mybir.dt.bfloat16, isOutput=False)
    posr_d = nc.declare_dram_parameter("posr", [P, TILES], mybir.dt.float32,
                                       isOutput=False)
    negposr_d = nc.declare_dram_parameter("negposr", [P, TILES],
                                          mybir.dt.float32, isOutput=False)
    cnt_d = nc.declare_dram_parameter("cntf", [P, TILES], mybir.dt.float32,
                                      isOutput=False)
    out = nc.declare_dram_parameter("out", [ROWS, 2], mybir.dt.float32,
                                    isOutput=True)

    IDXW = S // 16 // (NCALLS // IDX_GROUP_CALLS + (1 if NCALLS %
                       IDX_GROUP_CALLS else 0)) if False else CALL_IDXS // 16
    CW = CALL_IDXS // 16          # idx free-dim per call (64)

    with TileContext(nc) as tc:
        with (
            tc.tile_pool(name="big", bufs=1) as big,
            tc.tile_pool(name="idxp", bufs=2) as idxp,
            tc.tile_pool(name="smallp", bufs=8) as smallp,
        ):
            nc.gpsimd.load_library(library_config.mlp)
            nreg = nc.gpsimd.to_reg(CALL_IDXS)

            # persistent buffers
            g4 = big.tile([P, COLS_PAD * RLANES], mybir.dt.bfloat16, tag="g4")
            ohb = big.tile([P, COLS_PAD * RLANES], mybir.dt.bfloat16, tag="oh")
            nc.sync.dma_start(out=ohb[:], in_=oh_d[:])
            posr_t = smallp.tile([P, TILES], mybir.dt.float32, tag="posr")
            nc.sync.dma_start(out=posr_t[:], in_=posr_d[:])
            negposr_t = smallp.tile([P, TILES], mybir.dt.float32, tag="negposr")
            nc.sync.dma_start(out=negposr_t[:], in_=negposr_d[:])
            cnt_t = smallp.tile([P, TILES], mybir.dt.float32, tag="cnt")
            nc.sync.dma_start(out=cnt_t[:], in_=cnt_d[:])

            # gather: NCALLS x 1024-idx dma_gather into g4 slices;
            # idx staged in groups to keep HWDGE transfers big
            GROUPS = (NCALLS + IDX_GROUP_CALLS - 1) // IDX_GROUP_CALLS
            for g in range(GROUPS):
                c0 = g * IDX_GROUP_CALLS
                c1 = min(c0 + IDX_GROUP_CALLS, NCALLS)
                idxg = idxp.tile([P, (c1 - c0) * CW], mybir.dt.int16, tag="idxg")
                nc.sync.dma_start(
                    out=idxg[:], in_=idx_d[:, c0 * CW:c1 * CW])
                for i in range(c0, c1):
                    gview = g4[:, i * CALL_COLS * RLANES:
                               (i + 1) * CALL_COLS * RLANES] \
                        .rearrange("p (c q) -> p c q", q=RLANES)
                    _dma_gather_raw(
                        nc, gview, posrows[:, 0:RLANES],
                        idxg[:, (i - c0) * CW:(i - c0 + 1) * CW],
                        CALL_IDXS, elem_size=RLANES, elem_step=ROWW, nreg=nreg,
                        queue_num=i % 4)

            # per-tile: d = g4 - pos_dst ; e = d*oh ; s = reduce(|e|)
            s_t = smallp.tile([P, TILES], mybir.dt.float32, tag="s")
            Kmax = int(K_t.max())
            off = 0
            for t in range(TILES):
                K = int(K_t[t])
                w0, w1 = off * RLANES, (off + K) * RLANES
                d_t = smallp.tile([P, RLANES * Kmax], mybir.dt.bfloat16, tag="d")
                nc.scalar.add(
                    out=d_t[:, :RLANES * K], in_=g4[:, w0:w1],
                    add=negposr_t[:, t:t + 1])
                nc.vector.tensor_tensor(
                    out=d_t[:, :RLANES * K], in0=d_t[:, :RLANES * K],
                    in1=ohb[:, w0:w1], op=mybir.AluOpType.mult)
                nc.vector.tensor_reduce(
                    out=s_t[:, t:t + 1], in_=d_t[:, :RLANES * K],
                    axis=mybir.AxisListType.X,
                    op=mybir.AluOpType.add, apply_absolute_value=True)
                off += K

            # mean = s / max(cnt, 1); out = [pos, mean]
            nc.vector.tensor_scalar_max(out=cnt_t[:], in0=cnt_t[:], scalar1=1.0)
            rec_t = smallp.tile([P, TILES], mybir.dt.float32, tag="rec")
            nc.vector.reciprocal(out=rec_t[:], in_=cnt_t[:])
            o_t = smallp.tile([P, 2 * TILES], mybir.dt.float32, tag="o")
            o3 = o_t[:].rearrange("p (t two) -> p t two", two=2)
            nc.vector.tensor_copy(out=o3[:, :, 0:1], in_=posr_t[:])
            nc.vector.tensor_tensor(
                out=o3[:, :, 1:2], in0=s_t[:], in1=rec_t[:],
                op=mybir.AluOpType.mult)
            # out DRAM [ROWS, 2] = [TILES*P, 2]; row t*128+p = o_t[p, 2t:2t+2]
            nc.sync.dma_start(
                out=out[:].rearrange("(t p) two -> p t two", p=P), in_=o_t[:])

    _split_sync_waits(nc)
    mybir.codegen_inst_isa_subclasses(nc)
    return nc


def kernel(h, src, dst):
    h = np.asarray(h)
    src = np.asarray(src)
    dst = np.asarray(dst)
    in_maps, meta = _host_prep(h, src, dst)
    nc = _build_program(meta)
    res = run_bass_kernel_spmd(nc, in_maps, list(range(NC)))
    N, NPC, TILES, node_ids = (meta["N"], meta["NPC"], meta["TILES"],
                               meta["node_ids"])
    final = np.empty((N, 2), np.float32)
    for c in range(NC):
        o = res.results[c]["out"][:NPC]
        final[node_ids[c]] = o
    # exact pos in column 0 (bf16 staging only affected the device's copy)
    final[:, 0] = np.ascontiguousarray(h[:, 0], dtype=np.float32)
    return final
